# revision 1
# baseline (speedup 1.0000x reference)
"""MoE (top-2 of 8 experts + shared expert) Trainium2 kernel, 8 NeuronCores.

Strategy
--------
Host (numpy): router matmul + top-2 + softmax gates (0.01% of FLOPs), token
dispatch (gather by expert), final combine (concat shared slices, scatter-add
gated expert outputs).

Device (8 cores, SPMD): core c computes
  1. expert c's FFN over the tokens routed to it (padded to capacity C):
     h = x @ w13.T ; a = silu(h[:, :F]) * h[:, F:] ; y = a @ w2.T
     The per-token gate g is folded into the *up* projection input on the
     host (g*x) so no broadcast multiply is needed on device:
     g*y == (silu(x@Wg.T) * ((g*x)@Wu.T)) @ w2.T.
  2. the shared-expert FFN for token slice [c*512, (c+1)*512).

All matmuls run as float32r (TF32: fp32 range, 10-bit mantissa, fp32
accumulation in PSUM) at full PE rate. Inputs are pre-rounded to TF32 on the
host (round-to-nearest-even), so no on-device casts are needed.

Everything is kept feature-major ("transposed": [feature, token]) so the
contraction dim is always the SBUF partition dim.
"""

import math

import numpy as np

import concourse.bass as bass
import concourse.mybir as mybir
import concourse.tile as tile
from concourse.bass_utils import run_bass_kernel_spmd

T, D, E, F, FS, TOP_K = 4096, 2048, 8, 4096, 4096, 2
NCORES = 8
P = 128
TS = T // NCORES  # shared-expert tokens per core
DK = D // P

F32 = mybir.dt.float32
MMDT = mybir.dt.float32r


def _split_multiwaits(nc):
    """This toolchain's walrus allows at most ONE fused sem-wait per
    instruction, but TileContext's assign_waits can emit several. Split the
    extras into standalone InstEventSemaphore instructions inserted
    immediately before the owning instruction on the same engine."""
    for fn in nc.m.functions:
        for bb in fn.blocks:
            insts = list(bb.instructions)
            out = []
            changed = False
            for inst in insts:
                si = inst.sync_info
                waits = list(si.on_wait) if (si and si.on_wait) else []
                if len(waits) > 1:
                    for w in waits[:-1]:
                        out.append(
                            mybir.InstEventSemaphore(
                                name=nc.get_next_instruction_name(),
                                engine=inst.engine,
                                ins=[],
                                outs=[],
                                sync_info=mybir.SyncInfo(on_wait=[w], on_update=[]),
                            )
                        )
                    inst.sync_info = mybir.SyncInfo(
                        on_wait=[waits[-1]], on_update=list(si.on_update)
                    )
                    changed = True
                out.append(inst)
            if changed:
                bb.instructions = out


def round_tf32(x: np.ndarray) -> np.ndarray:
    """Round fp32 to TF32 (10-bit mantissa), round-to-nearest-even."""
    u = np.ascontiguousarray(x, dtype=np.float32).view(np.uint32).copy()
    low = u & np.uint32(0x1FFF)
    bit13 = (u >> np.uint32(13)) & np.uint32(1)
    round_up = (low > 0x1000) | ((low == 0x1000) & (bit13 == 1))
    u = (u & ~np.uint32(0x1FFF)) + (round_up.astype(np.uint32) << np.uint32(13))
    return u.view(np.float32)


def _emit_ffn(nc, pools, xg_d, xu_d, w13_d, w2_d, out_d, n_tok, ct, fdim):
    """Emit one SwiGLU FFN: out[D, n_tok] = swiglu(x, w13) @ w2, transposed
    layouts everywhere. xg_d/xu_d: [DK, P, n_tok] gate/up inputs (may be the
    same tensor). w13_d: [DK, P, 2*fdim]. w2_d: [fdim//P, P, D].
    out_d: [DK, P, n_tok] (fp32)."""
    xp, wp, w2p, atp, op, ps1, ps2 = pools
    FT = fdim // P
    n_ch = n_tok // ct
    nspl = 1 if ct <= 512 else 2
    spl = ct // nspl
    G = 4 // nspl  # d-tiles per GEMM2 psum group

    silu = mybir.ActivationFunctionType.Silu
    xg_ap = xg_d[:].rearrange("k p c -> p k c")
    xu_ap = xu_d[:].rearrange("k p c -> p k c")
    w13_ap = w13_d[:].rearrange("k p f -> p k f")
    out_ap = out_d[:].rearrange("k p c -> p k c")

    for ch in range(n_ch):
        c0 = ch * ct
        xg = xp.tile([P, DK, ct], MMDT, tag="x", name="xg")
        nc.sync.dma_start(out=xg, in_=xg_ap[:, :, c0 : c0 + ct])
        aT = atp.tile([P, FT, ct], MMDT, tag="aT", name="aT")

        # gate half: aT = silu(x @ w13[:fdim].T)
        WFB = 2  # f-tiles per w13 DMA block (>=1KB per-partition lines)
        for ftp in range(FT // WFB):
            wt = wp.tile([P, DK, WFB * P], MMDT, tag="w13", name="wt")
            f0 = ftp * WFB * P
            nc.sync.dma_start(out=wt, in_=w13_ap[:, :, f0 : f0 + WFB * P])
            for fi in range(WFB):
                ft = ftp * WFB + fi
                for s in range(nspl):
                    pt = ps1.tile([P, 512], F32, tag="ps", name="pt")
                    for k in range(DK):
                        nc.tensor.matmul(
                            pt[:, :spl],
                            wt[:, k, fi * P : (fi + 1) * P],
                            xg[:, k, s * spl : (s + 1) * spl],
                            start=(k == 0),
                            stop=(k == DK - 1),
                        )
                    nc.scalar.activation(
                        out=aT[:, ft, s * spl : (s + 1) * spl],
                        in_=pt[:, :spl],
                        func=silu,
                    )

        # up half: aT *= (g*x) @ w13[fdim:].T
        if xu_d is xg_d:
            xu = xg
        else:
            xu = xp.tile([P, DK, ct], MMDT, tag="x", name="xu")
            nc.sync.dma_start(out=xu, in_=xu_ap[:, :, c0 : c0 + ct])
        for ftp in range(FT // WFB):
            wt = wp.tile([P, DK, WFB * P], MMDT, tag="w13", name="wt")
            f0 = (FT + ftp * WFB) * P
            nc.sync.dma_start(out=wt, in_=w13_ap[:, :, f0 : f0 + WFB * P])
            for fi in range(WFB):
                ft = ftp * WFB + fi
                for s in range(nspl):
                    pt = ps1.tile([P, 512], F32, tag="ps", name="pt")
                    for k in range(DK):
                        nc.tensor.matmul(
                            pt[:, :spl],
                            wt[:, k, fi * P : (fi + 1) * P],
                            xu[:, k, s * spl : (s + 1) * spl],
                            start=(k == 0),
                            stop=(k == DK - 1),
                        )
                    sl = aT[:, ft, s * spl : (s + 1) * spl]
                    nc.vector.tensor_mul(out=sl, in0=sl, in1=pt[:, :spl])

        # GEMM2: y[D, ct] = w2.T-contract over fdim, accumulated in PSUM
        for dg in range(DK // G):
            psy = [ps2.tile([P, 512], F32, tag="ps", name=f"psy{_i}") for _i in range(G * nspl)]
            for k in range(FT):
                w2t = w2p.tile([P, G * P], MMDT, tag="w2", name="w2t")
                nc.sync.dma_start(
                    out=w2t, in_=w2_d[:][k, :, dg * G * P : (dg + 1) * G * P]
                )
                for gi in range(G):
                    for s in range(nspl):
                        nc.tensor.matmul(
                            psy[gi * nspl + s][:, :spl],
                            w2t[:, gi * P : (gi + 1) * P],
                            aT[:, k, s * spl : (s + 1) * spl],
                            start=(k == 0),
                            stop=(k == FT - 1),
                        )
            for gi in range(G):
                ot = op.tile([P, ct], F32, tag="o", name="ot")
                for s in range(nspl):
                    nc.vector.tensor_copy(
                        out=ot[:, s * spl : (s + 1) * spl],
                        in_=psy[gi * nspl + s][:, :spl],
                    )
                nc.sync.dma_start(
                    out=out_ap[:, dg * G + gi, c0 : c0 + ct], in_=ot
                )


def build_program(C, CT):
    nc = bass.Bass()
    xeT = nc.dram_tensor("xeT", [DK, P, C], MMDT, kind="ExternalInput")
    xegT = nc.dram_tensor("xegT", [DK, P, C], MMDT, kind="ExternalInput")
    w13T = nc.dram_tensor("w13T", [DK, P, 2 * F], MMDT, kind="ExternalInput")
    w2T = nc.dram_tensor("w2T", [F // P, P, D], MMDT, kind="ExternalInput")
    xsT = nc.dram_tensor("xsT", [DK, P, TS], MMDT, kind="ExternalInput")
    sw13T = nc.dram_tensor("sw13T", [DK, P, 2 * FS], MMDT, kind="ExternalInput")
    sw2T = nc.dram_tensor("sw2T", [FS // P, P, D], MMDT, kind="ExternalInput")
    yeT = nc.dram_tensor("yeT", [DK, P, C], F32, kind="ExternalOutput")
    ysT = nc.dram_tensor("ysT", [DK, P, TS], F32, kind="ExternalOutput")

    with tile.TileContext(nc) as tc:
        with (
            tc.tile_pool(name="xp", bufs=2) as xp,
            tc.tile_pool(name="wp", bufs=4) as wp,
            tc.tile_pool(name="w2p", bufs=4) as w2p,
            tc.tile_pool(name="atp", bufs=1) as atp,
            tc.tile_pool(name="op", bufs=3) as op,
            tc.tile_pool(name="ps", bufs=8, space="PSUM") as ps,
        ):
            pools = (xp, wp, w2p, atp, op, ps, ps)
            _emit_ffn(nc, pools, xeT, xegT, w13T, w2T, yeT, C, CT, F)
            _emit_ffn(nc, pools, xsT, xsT, sw13T, sw2T, ysT, TS, TS, FS)
    _split_multiwaits(nc)
    return nc


_PROG_CACHE = {}

# test harnesses may override, e.g. {"trace": True, "trace_cores": [...]}
RUN_KWARGS = {}


def _get_program(C, CT):
    key = (C, CT)
    if key not in _PROG_CACHE:
        _PROG_CACHE[key] = build_program(C, CT)
    return _PROG_CACHE[key]


def kernel(x, router_DE, w13, w2, shared_w13, shared_w2):
    x = np.asarray(x, dtype=np.float32)
    router_DE = np.asarray(router_DE, dtype=np.float32)
    w13 = np.asarray(w13, dtype=np.float32)
    w2 = np.asarray(w2, dtype=np.float32)
    shared_w13 = np.asarray(shared_w13, dtype=np.float32)
    shared_w2 = np.asarray(shared_w2, dtype=np.float32)

    # ---- routing (host) ----
    logits = x @ router_DE  # [T, E]
    top_idx = np.argsort(-logits, axis=1, kind="stable")[:, :TOP_K]  # [T, K]
    top_vals = np.take_along_axis(logits, top_idx, axis=1)
    ex = np.exp(top_vals - top_vals.max(axis=1, keepdims=True))
    gates = (ex / ex.sum(axis=1, keepdims=True)).astype(np.float32)

    toks_per_e, gates_per_e = [], []
    for e in range(E):
        hit = top_idx == e  # [T, K]
        tok_mask = hit.any(axis=1)
        toks = np.nonzero(tok_mask)[0]
        g = (gates * hit).sum(axis=1)[toks].astype(np.float32)
        toks_per_e.append(toks)
        gates_per_e.append(g)

    max_cnt = max(len(t) for t in toks_per_e)
    # pick chunk count/size minimizing PE cycles: k chunks of CT tokens,
    # fp32r matmul cost ~ (N + 111) cycles, full rate needs 256 <= N <= 512
    best = None
    for k in range(1, 12):
        ct = math.ceil(max_cnt / k / 8) * 8
        if ct > 512:
            continue
        ct = max(ct, 256)
        cost = k * (ct + 111)
        if best is None or cost < best[0]:
            best = (cost, k, ct)
    _, k, CT = best
    C = k * CT

    # ---- host-side shard prep ----
    xT = np.ascontiguousarray(x.T)  # [D, T]
    xT_r = round_tf32(xT)
    sw13T = round_tf32(np.ascontiguousarray(shared_w13.T)).reshape(DK, P, 2 * FS)
    sw2T = round_tf32(np.ascontiguousarray(shared_w2.T)).reshape(FS // P, P, D)

    in_maps = []
    for c in range(NCORES):
        toks, g = toks_per_e[c], gates_per_e[c]
        cnt = len(toks)
        xe = np.zeros((D, C), np.float32)
        xe[:, :cnt] = xT_r[:, toks]
        xeg = np.zeros((D, C), np.float32)
        xeg[:, :cnt] = round_tf32(xT[:, toks] * g[None, :])
        in_maps.append(
            {
                "xeT": xe.reshape(DK, P, C),
                "xegT": xeg.reshape(DK, P, C),
                "w13T": round_tf32(np.ascontiguousarray(w13[c].T)).reshape(
                    DK, P, 2 * F
                ),
                "w2T": round_tf32(np.ascontiguousarray(w2[c].T)).reshape(
                    F // P, P, D
                ),
                "xsT": np.ascontiguousarray(xT_r[:, c * TS : (c + 1) * TS]).reshape(
                    DK, P, TS
                ),
                "sw13T": sw13T,
                "sw2T": sw2T,
            }
        )

    nc = _get_program(C, CT)
    res = run_bass_kernel_spmd(nc, in_maps, list(range(NCORES)), **RUN_KWARGS)
    kernel.last_result = res

    # ---- combine (host) ----
    out = np.empty((T, D), np.float32)
    for c in range(NCORES):
        out[c * TS : (c + 1) * TS] = res.results[c]["ysT"].reshape(D, TS).T
    for c in range(NCORES):
        toks = toks_per_e[c]
        ye = res.results[c]["yeT"].reshape(D, C)
        out[toks] += ye[:, : len(toks)].T
    return out



# revision 2
# speedup vs baseline: 2.0186x; 2.0186x over previous
"""MoE (top-2 of 8 experts + shared expert) Trainium2 kernel, 8 NeuronCores.

Strategy
--------
Host (numpy): router matmul + top-2 + softmax gates (0.01% of FLOPs), token
dispatch (gather by expert), final combine (concat shared slices, scatter-add
gated expert outputs).

Device (8 cores, SPMD): core c computes
  1. expert c's FFN over the tokens routed to it (padded to capacity C)
  2. the shared-expert FFN for token slice [c*512, (c+1)*512).

All tensors are bf16 (fp32 PSUM accumulation). The routing gate g is applied
on the *output* copy (PSUM -> SBUF multiply against a broadcast gate tile), so
x is sent once and no extra device work is needed.

Loop structure keeps weights resident: every w13/w2 tile is DMA'd exactly once
and all token chunks are processed against it (the token-chunk loop is INSIDE
the weight loop; activations aT for all chunks stay in SBUF). This cuts HBM
traffic from ~400 MB/core (fp32, weights re-streamed per chunk) to ~120
MB/core, far under the PE time.

Everything is feature-major ("transposed": [feature, token]) so the
contraction dim is always the SBUF partition dim. w13 rows are interleaved
per 128-row tile (gate t at 2t, up t at 2t+1) so one weight block carries a
(gate, up) pair.
"""

import math

import ml_dtypes
import numpy as np

import concourse.bass as bass
import concourse.mybir as mybir
import concourse.tile as tile
from concourse.bass_utils import run_bass_kernel_spmd

T, D, E, F, FS, TOP_K = 4096, 2048, 8, 4096, 4096, 2
NCORES = 8
P = 128
TS = T // NCORES  # shared-expert tokens per core
DK = D // P  # 16
FT = F // P  # 32
DG = 4  # d-tiles per GEMM2 psum group (512 outputs)

F32 = mybir.dt.float32
BF16 = mybir.dt.bfloat16
BF = ml_dtypes.bfloat16


def _split_multiwaits(nc):
    """This toolchain's walrus allows at most ONE fused sem-wait per
    instruction, but TileContext's assign_waits can emit several. Split the
    extras into standalone InstEventSemaphore instructions inserted
    immediately before the owning instruction on the same engine."""
    for fn in nc.m.functions:
        for bb in fn.blocks:
            insts = list(bb.instructions)
            out = []
            changed = False
            for inst in insts:
                si = inst.sync_info
                waits = list(si.on_wait) if (si and si.on_wait) else []
                if len(waits) > 1:
                    for w in waits[:-1]:
                        out.append(
                            mybir.InstEventSemaphore(
                                name=nc.get_next_instruction_name(),
                                engine=inst.engine,
                                ins=[],
                                outs=[],
                                sync_info=mybir.SyncInfo(on_wait=[w], on_update=[]),
                            )
                        )
                    inst.sync_info = mybir.SyncInfo(
                        on_wait=[waits[-1]], on_update=list(si.on_update)
                    )
                    changed = True
                out.append(inst)
            if changed:
                bb.instructions = out


def _emit_ffn(nc, pools, x_d, w13_d, w2_d, out_d, g_d, chunks, fdim):
    """One SwiGLU FFN, transposed layouts, weights streamed exactly once.

    x_d: [DK, P, n_tok] bf16. w13_d: [DK, P, 2*fdim] bf16, f-tiles interleaved
    (gate tile t at columns 2t*P, up tile t at (2t+1)*P). w2_d:
    [fdim//P, P, D] bf16. out_d: [DK, P, n_tok] f32. g_d: [P, n_tok] f32
    broadcast gate (None => plain copy out). chunks: [(c0, ct)], ct <= 512.
    """
    xp, wp, w2p, atp, op, gp, ps = pools
    FTl = fdim // P
    n_tok = chunks[-1][0] + chunks[-1][1]
    silu = mybir.ActivationFunctionType.Silu

    x_ap = x_d[:].rearrange("k p c -> p k c")
    w13_ap = w13_d[:].rearrange("k p f -> p k f")
    out_ap = out_d[:].rearrange("k p c -> p k c")

    xt = xp.tile([P, DK, n_tok], BF16, tag="x", name="xt")
    nc.sync.dma_start(out=xt, in_=x_ap)
    if g_d is not None:
        gt = gp.tile([P, n_tok], F32, tag="g", name="gt")
        nc.sync.dma_start(out=gt, in_=g_d[:])
    aT = atp.tile([P, FTl, n_tok], BF16, tag="aT", name="aT")

    # ---- GEMM1: aT[f, t] = silu(x@Wg.T) * (x@Wu.T), per interleaved block
    for b in range(FTl):
        wt = wp.tile([P, DK, 2 * P], BF16, tag="w13", name="wt")
        nc.sync.dma_start(out=wt, in_=w13_ap[:, :, 2 * b * P : 2 * (b + 1) * P])
        for c0, ct in chunks:
            pt = ps.tile([P, 512], F32, tag="ps", name="ptg")
            for k in range(DK):
                nc.tensor.matmul(
                    pt[:, :ct],
                    wt[:, k, 0:P],
                    xt[:, k, c0 : c0 + ct],
                    start=(k == 0),
                    stop=(k == DK - 1),
                )
            nc.scalar.activation(
                out=aT[:, b, c0 : c0 + ct], in_=pt[:, :ct], func=silu
            )
        for c0, ct in chunks:
            pt = ps.tile([P, 512], F32, tag="ps", name="ptu")
            for k in range(DK):
                nc.tensor.matmul(
                    pt[:, :ct],
                    wt[:, k, P : 2 * P],
                    xt[:, k, c0 : c0 + ct],
                    start=(k == 0),
                    stop=(k == DK - 1),
                )
            sl = aT[:, b, c0 : c0 + ct]
            nc.vector.tensor_mul(out=sl, in0=sl, in1=pt[:, :ct])

    # ---- GEMM2: y[d, t] = w2 @ aT, d in groups of DG tiles, w2 slice resident
    for dg in range(DK // DG):
        w2ts = []
        for k in range(FTl):
            w2t = w2p.tile([P, DG * P], BF16, tag="w2", name="w2t")
            nc.sync.dma_start(
                out=w2t, in_=w2_d[:][k, :, dg * DG * P : (dg + 1) * DG * P]
            )
            w2ts.append(w2t)
        for c0, ct in chunks:
            psy = [
                ps.tile([P, 512], F32, tag="ps", name=f"psy{i}") for i in range(DG)
            ]
            for k in range(FTl):
                for gi in range(DG):
                    nc.tensor.matmul(
                        psy[gi][:, :ct],
                        w2ts[k][:, gi * P : (gi + 1) * P],
                        aT[:, k, c0 : c0 + ct],
                        start=(k == 0),
                        stop=(k == FTl - 1),
                    )
            for gi in range(DG):
                ot = op.tile([P, 512], F32, tag="o", name="ot")
                if g_d is not None:
                    nc.vector.tensor_mul(
                        out=ot[:, :ct], in0=psy[gi][:, :ct], in1=gt[:, c0 : c0 + ct]
                    )
                else:
                    nc.vector.tensor_copy(out=ot[:, :ct], in_=psy[gi][:, :ct])
                nc.sync.dma_start(
                    out=out_ap[:, dg * DG + gi, c0 : c0 + ct], in_=ot[:, :ct]
                )


def build_program(C, CT):
    n_ch = C // CT
    chunks = [(i * CT, CT) for i in range(n_ch)]

    nc = bass.Bass()
    xeT = nc.dram_tensor("xeT", [DK, P, C], BF16, kind="ExternalInput")
    gE = nc.dram_tensor("gE", [P, C], F32, kind="ExternalInput")
    w13T = nc.dram_tensor("w13T", [DK, P, 2 * F], BF16, kind="ExternalInput")
    w2T = nc.dram_tensor("w2T", [F // P, P, D], BF16, kind="ExternalInput")
    xsT = nc.dram_tensor("xsT", [DK, P, TS], BF16, kind="ExternalInput")
    sw13T = nc.dram_tensor("sw13T", [DK, P, 2 * FS], BF16, kind="ExternalInput")
    sw2T = nc.dram_tensor("sw2T", [FS // P, P, D], BF16, kind="ExternalInput")
    yeT = nc.dram_tensor("yeT", [DK, P, C], F32, kind="ExternalOutput")
    ysT = nc.dram_tensor("ysT", [DK, P, TS], F32, kind="ExternalOutput")

    with tile.TileContext(nc) as tc:
        with (
            tc.tile_pool(name="xp", bufs=1) as xp,
            tc.tile_pool(name="wp", bufs=3) as wp,
            tc.tile_pool(name="w2p", bufs=FT + 4) as w2p,
            tc.tile_pool(name="atp", bufs=1) as atp,
            tc.tile_pool(name="op", bufs=3) as op,
            tc.tile_pool(name="gp", bufs=1) as gp,
            tc.tile_pool(name="ps", bufs=8, space="PSUM") as ps,
        ):
            pools = (xp, wp, w2p, atp, op, gp, ps)
            _emit_ffn(nc, pools, xeT, w13T, w2T, yeT, gE, chunks, F)
            _emit_ffn(nc, pools, xsT, sw13T, sw2T, ysT, None, [(0, TS)], FS)
    _split_multiwaits(nc)
    return nc


_PROG_CACHE = {}

# test harnesses may override, e.g. {"trace": True, "trace_cores": [...]}
RUN_KWARGS = {}


def _get_program(C, CT):
    key = (C, CT)
    if key not in _PROG_CACHE:
        _PROG_CACHE[key] = build_program(C, CT)
    return _PROG_CACHE[key]


def _interleave_w13(w13_e):
    """[2F', D] fp32 -> [DK, P, 2F'] bf16 with (gate, up) 128-row tiles
    interleaved along the output feature axis."""
    fdim = w13_e.shape[0] // 2
    ftl = fdim // P
    wg = w13_e[:fdim].reshape(ftl, P, -1)
    wu = w13_e[fdim:].reshape(ftl, P, -1)
    wi = np.stack([wg, wu], axis=1).reshape(2 * fdim, -1)  # interleaved rows
    return np.ascontiguousarray(wi.T.astype(BF)).reshape(DK, P, 2 * fdim)


def kernel(x, router_DE, w13, w2, shared_w13, shared_w2):
    x = np.asarray(x, dtype=np.float32)
    router_DE = np.asarray(router_DE, dtype=np.float32)
    w13 = np.asarray(w13, dtype=np.float32)
    w2 = np.asarray(w2, dtype=np.float32)
    shared_w13 = np.asarray(shared_w13, dtype=np.float32)
    shared_w2 = np.asarray(shared_w2, dtype=np.float32)

    # ---- routing (host) ----
    logits = x @ router_DE  # [T, E]
    top_idx = np.argsort(-logits, axis=1, kind="stable")[:, :TOP_K]  # [T, K]
    top_vals = np.take_along_axis(logits, top_idx, axis=1)
    ex = np.exp(top_vals - top_vals.max(axis=1, keepdims=True))
    gates = (ex / ex.sum(axis=1, keepdims=True)).astype(np.float32)

    toks_per_e, gates_per_e = [], []
    for e in range(E):
        hit = top_idx == e  # [T, K]
        toks = np.nonzero(hit.any(axis=1))[0]
        g = (gates * hit).sum(axis=1)[toks].astype(np.float32)
        toks_per_e.append(toks)
        gates_per_e.append(g)

    max_cnt = max(len(t) for t in toks_per_e)
    n_ch = max(1, math.ceil(max_cnt / 512))
    CT = math.ceil(max_cnt / n_ch / 8) * 8
    C = n_ch * CT

    # ---- host-side shard prep ----
    xTb = np.ascontiguousarray(x.T).astype(BF)  # [D, T] bf16
    sw13T = _interleave_w13(shared_w13)
    sw2T = np.ascontiguousarray(shared_w2.T.astype(BF)).reshape(FS // P, P, D)

    in_maps = []
    for c in range(NCORES):
        toks, g = toks_per_e[c], gates_per_e[c]
        cnt = len(toks)
        xe = np.zeros((D, C), BF)
        xe[:, :cnt] = xTb[:, toks]
        ge = np.zeros((P, C), np.float32)
        ge[:, :cnt] = g[None, :]
        in_maps.append(
            {
                "xeT": xe.reshape(DK, P, C),
                "gE": ge,
                "w13T": _interleave_w13(w13[c]),
                "w2T": np.ascontiguousarray(w2[c].T.astype(BF)).reshape(
                    F // P, P, D
                ),
                "xsT": np.ascontiguousarray(
                    xTb[:, c * TS : (c + 1) * TS]
                ).reshape(DK, P, TS),
                "sw13T": sw13T,
                "sw2T": sw2T,
            }
        )

    nc = _get_program(C, CT)
    res = run_bass_kernel_spmd(nc, in_maps, list(range(NCORES)), **RUN_KWARGS)
    kernel.last_result = res

    # ---- combine (host) ----
    out = np.empty((T, D), np.float32)
    for c in range(NCORES):
        out[c * TS : (c + 1) * TS] = res.results[c]["ysT"].reshape(D, TS).T
    for c in range(NCORES):
        toks = toks_per_e[c]
        ye = res.results[c]["yeT"].reshape(D, C)
        out[toks] += ye[:, : len(toks)].T
    return out
